# revision 27
# baseline (speedup 1.0000x reference)
"""Multi-head causal attention (B=4, S=2048, D=1024, H=16) on 8 Trainium2 cores.

Sharding: core c = (batch b = c//2, head-group j = c%2). Each core computes
one batch and 8 heads (a 512-wide slice of D) entirely on-chip:

  Q^T = Wq_j^T @ x_b^T          [512, 2048]   (bf16, no on-chip transposes)
  K^T = Wk_j^T @ x_b^T          [512, 2048]
  V   = x_b @ Wv_j              [2048, 512]   (k on partitions, + ones column)
  per head h, q-block of 512:
    S^T tile = K_h @ Q_h^T      [128k, 512q]  (contraction over hd=64)
    P^T = exp(S^T / 8)          (no max subtraction needed: scores ~ N(0,1))
    causal: skip/trim tiles above the diagonal, triangle-mask diag tiles
    [ctx^T | rowsum] += [V_h | 1]^T @ P^T     [65, 512] PSUM accumulation
    ctx^T /= rowsum             (reciprocal + partition_broadcast)
  out_partial = ctx^T.T @ Wo_j  [2048, 1024]  (fp32, DMA from PSUM)

Host sums the two head-group partials per batch and adds bo.
"""

import sys

if "/opt/trn_rl_repo" not in sys.path:
    sys.path.insert(0, "/opt/trn_rl_repo")

import numpy as np
import ml_dtypes

import concourse.bacc as bacc
import concourse.tile as tile
from concourse import mybir
from concourse.alu_op_type import AluOpType
from concourse.bass_utils import run_bass_kernel_spmd

B, S, D, H = 4, 2048, 1024, 16
HD = D // H            # 64
JG = 2                 # head-group shards
HPC = H // JG          # 8 heads per core
DG = D // JG           # 512-wide D slice per core
QB = 512               # q-block width
NQB = S // QB          # 4
NKT = S // 128         # 16 k partition-tiles
BF16 = mybir.dt.bfloat16
F32 = mybir.dt.float32
EXP = mybir.ActivationFunctionType.Exp

_CACHE = {}


def _build():
    nc = bacc.Bacc("TRN2", target_bir_lowering=False, debug=False)
    xT = nc.dram_tensor("xT", [D, S], BF16, kind="ExternalInput").ap()
    wq = nc.dram_tensor("wq", [D, DG], BF16, kind="ExternalInput").ap()
    wk = nc.dram_tensor("wk", [D, DG], BF16, kind="ExternalInput").ap()
    wv = nc.dram_tensor("wv", [D, DG], BF16, kind="ExternalInput").ap()
    wo = nc.dram_tensor("wo", [DG, D], BF16, kind="ExternalInput").ap()
    out = nc.dram_tensor("out", [S, D], F32, kind="ExternalOutput").ap()

    xT_r = xT.rearrange("(t p) n -> t p n", p=128)   # [8, 128, 2048]
    wq_r = wq.rearrange("(t p) n -> t p n", p=128)   # [8, 128, 512]
    wk_r = wk.rearrange("(t p) n -> t p n", p=128)
    wv_r = wv.rearrange("(t p) n -> t p n", p=128)
    wo_r = wo.rearrange("(t p) n -> t p n", p=128)   # [4, 128, 1024]

    with tile.TileContext(nc) as tc:
        with (
            tc.tile_pool(name="pers", bufs=1) as pers,
            tc.tile_pool(name="pT", bufs=8) as ppool,
            tc.tile_pool(name="vec", bufs=3) as vpool,
            tc.tile_pool(name="psmm", bufs=2, space="PSUM") as psmm,
            tc.tile_pool(name="pssc", bufs=4, space="PSUM") as pssc,
            tc.tile_pool(name="psctx", bufs=2, space="PSUM") as psctx,
        ):
            xt = [pers.tile([128, S], BF16, tag=f"xt{d}", name=f"xt{d}") for d in range(8)]
            wqt = [pers.tile([128, DG], BF16, tag=f"wq{d}", name=f"wq{d}") for d in range(8)]
            wkt = [pers.tile([128, DG], BF16, tag=f"wk{d}", name=f"wk{d}") for d in range(8)]
            wvt = [pers.tile([128, DG], BF16, tag=f"wv{d}", name=f"wv{d}") for d in range(8)]
            wot = [pers.tile([128, D], BF16, tag=f"wo{d}", name=f"wo{d}") for d in range(4)]
            qT = [pers.tile([128, S], BF16, tag=f"qT{m}", name=f"qT{m}") for m in range(4)]
            kT = [pers.tile([128, S], BF16, tag=f"kT{m}", name=f"kT{m}") for m in range(4)]
            # V with an extra all-ones column per head: [k, 8*(64+1)]
            vt = [pers.tile([128, HPC * (HD + 1)], BF16, tag=f"v{k}", name=f"v{k}")
                  for k in range(NKT)]
            cT = [pers.tile([128, S], BF16, tag=f"cT{m}", name=f"cT{m}") for m in range(4)]
            mask = pers.tile([128, 128], BF16, tag="mask", name="mask")

            # column-chunked x loads, interleaved with the K/Q weights in
            # d-order so the first score-chunk matmul (needs only wkt[0] +
            # xt[0] cols 0:512) starts ~1us in
            for d in range(8):
                nc.sync.dma_start(xt[d][:, 0:512], xT_r[d][:, 0:512])
                nc.sync.dma_start(wkt[d][:], wk_r[d])
                nc.sync.dma_start(wqt[d][:], wq_r[d])
            for d in range(8):
                nc.sync.dma_start(wvt[d][:], wv_r[d])
            for n in range(1, 4):
                for d in range(8):
                    nc.sync.dma_start(xt[d][:, n * 512:(n + 1) * 512],
                                      xT_r[d][:, n * 512:(n + 1) * 512])
            for d in range(4):
                nc.sync.dma_start(wot[d][:], wo_r[d])

            # triangle mask: keep (1.0) where col >= row, else 0
            nc.gpsimd.memset(mask[:], 1.0)
            nc.gpsimd.affine_select(
                out=mask[:], in_=mask[:], compare_op=AluOpType.is_ge,
                fill=0.0, base=0, pattern=[[1, 128]], channel_multiplier=-1,
            )
            for k in range(NKT):
                ones_cols = vt[k].rearrange("p (h c) -> p h c", c=HD + 1)
                nc.gpsimd.memset(ones_cols[:, :, HD:HD + 1], 1.0)

            # ---- emission helpers ----
            # Filler generators emit one PE matmul per yield; the filler
            # queue drip-feeds them into attention islands so projection
            # work rides in the PE slack while ACT (exp) stays saturated.
            def proj_chunk(wt, dst, m, n):
                """One [128,512] tile of Q^T / K^T (m-tile of D-slice rows,
                n-chunk of sequence columns)."""
                ps = psmm.tile([128, 512], F32, tag="mm", name="mm")
                for d in range(8):
                    nc.tensor.matmul(
                        ps[:],
                        wt[d][:, m * 128:(m + 1) * 128],
                        xt[d][:, n * 512:(n + 1) * 512],
                        start=(d == 0), stop=(d == 7),
                    )
                    if d == 7:
                        nc.vector.tensor_copy(
                            dst[m][:, n * 512:(n + 1) * 512], ps[:])
                    yield

            def v_chunk(k):
                """One [128 k-rows, 512] tile of V = x @ Wv (strided into the
                65-column-per-head layout)."""
                ps = psmm.tile([128, 512], F32, tag="mm", name="mm")
                for d in range(8):
                    nc.tensor.matmul(
                        ps[:],
                        xt[d][:, k * 128:(k + 1) * 128],
                        wvt[d][:],
                        start=(d == 0), stop=(d == 7),
                    )
                    if d == 7:
                        nc.vector.tensor_copy(
                            vt[k].rearrange(
                                "p (h c) -> p h c", c=HD + 1)[:, :, 0:HD],
                            ps[:].rearrange("p (h c) -> p h c", c=HD),
                        )
                    yield

            def outproj(qt, n2):
                ps = psmm.tile([128, 512], F32, tag="mm", name="mm")
                for dt in range(4):
                    nc.tensor.matmul(
                        ps[:],
                        cT[dt][:, qt * 128:(qt + 1) * 128],
                        wot[dt][:, n2 * 512:(n2 + 1) * 512],
                        start=(dt == 0), stop=(dt == 3),
                    )
                    if dt == 3:
                        ot = ppool.tile([128, 512], F32, tag="ot", name="ot",
                                        bufs=3)
                        nc.vector.tensor_copy(ot[:], ps[:])
                        nc.sync.dma_start(
                            out[qt * 128:(qt + 1) * 128,
                                n2 * 512:(n2 + 1) * 512],
                            ot[:])
                    yield

            class FillerQueue:
                def __init__(self):
                    self.gens = []
                    self.cur = 0

                def add(self, gen):
                    self.gens.append(gen)
                    return len(self.gens) - 1

                def advance(self, units):
                    while units > 0 and self.cur < len(self.gens):
                        try:
                            next(self.gens[self.cur])
                            units -= 1
                        except StopIteration:
                            self.cur += 1

                def drain_through(self, idx):
                    while self.cur <= idx:
                        try:
                            next(self.gens[self.cur])
                        except StopIteration:
                            self.cur += 1

                def drain_all(self):
                    self.drain_through(len(self.gens) - 1)

            def attn_pair(qb, t, fq, prereq, pv_prereq=None):
                """Attention for head pair (2t, 2t+1) on q-block qb.

                The two heads sit at SBUF partitions 0-63 / 64-127 of tile t,
                so their K=64 score matmuls land on disjoint PE row groups
                and run concurrently. PV matmuls trail by one k-step, and
                ~3 filler (projection) matmuls ride in each step's PE
                slack."""
                fq.drain_through(prereq)
                heads = (2 * t, 2 * t + 1)
                nk = 4 * qb + 4
                ctxs = {h: psctx.tile([65, 512], F32, tag="ctx", name="ctx")
                        for h in heads}

                def score(kt, h):
                    o = max(0, (kt - 4 * qb) * 128)
                    hp = (h % 2) * 64
                    sc = pssc.tile([128, 512], F32, tag="sc", name="sc")
                    nc.tensor.matmul(
                        sc[:, o:],
                        kT[t][hp:hp + 64, kt * 128:(kt + 1) * 128],
                        qT[t][hp:hp + 64, qb * 512 + o:(qb + 1) * 512],
                        start=True, stop=True,
                    )
                    pt = ppool.tile([128, 512], BF16, tag="pT", name="pT")
                    nc.scalar.activation(pt[:, o:], sc[:, o:], EXP,
                                         scale=0.125)
                    if kt >= 4 * qb:
                        nc.vector.tensor_tensor(
                            pt[:, o:o + 128], pt[:, o:o + 128], mask[:],
                            AluOpType.mult)
                    return kt, o, pt

                def pv(h, kt, o, pt):
                    nc.tensor.matmul(
                        ctxs[h][:, o:],
                        vt[kt][:, h * (HD + 1):(h + 1) * (HD + 1)],
                        pt[:, o:],
                        start=(kt == 0), stop=(kt == nk - 1),
                        skip_group_check=True,
                    )

                pend = []
                for kt in range(nk):
                    cur = [(h, *score(kt, h)) for h in heads]
                    pend.append(cur)
                    if len(pend) > 1:
                        if pv_prereq is not None:
                            fq.drain_through(pv_prereq)
                            pv_prereq = None
                        for args in pend.pop(0):
                            pv(*args)
                    fq.advance(1 if qb == 0 else 2)
                for step in pend:
                    for args in step:
                        pv(*args)

                for h in heads:
                    hp = (h % 2) * 64
                    rc = vpool.tile([1, 512], F32, tag="rc", name="rc")
                    nc.vector.reciprocal(rc[:], ctxs[h][64:65, :])
                    bc = vpool.tile([64, 512], F32, tag="bc", name="bc")
                    nc.gpsimd.partition_broadcast(bc[:], rc[:])
                    nc.vector.tensor_tensor(
                        cT[t][hp:hp + 64, qb * 512:(qb + 1) * 512],
                        ctxs[h][0:64, :], bc[:], AluOpType.mult)

            # ---- interleaved emission via the filler queue ----
            fq = FillerQueue()
            pre = {}
            # scores of island (0,0) gate ACT start: K00/Q00 first, V after
            # (the first PV only fires one k-step later)
            fq.add(proj_chunk(wkt, kT, 0, 0))
            pre[(0, 0)] = fq.add(proj_chunk(wqt, qT, 0, 0))
            for k in range(4):
                pv_pre_00 = fq.add(v_chunk(k))
            for t in range(1, 4):
                fq.add(proj_chunk(wkt, kT, t, 0))
                pre[(0, t)] = fq.add(proj_chunk(wqt, qT, t, 0))
            for qb in range(1, NQB):
                fq.add(proj_chunk(wkt, kT, 0, qb))
                q0 = fq.add(proj_chunk(wqt, qT, 0, qb))
                for k in range(4 * qb, 4 * qb + 4):
                    pre[(qb, 0)] = fq.add(v_chunk(k))
                for t in range(1, 4):
                    fq.add(proj_chunk(wkt, kT, t, qb))
                    pre[(qb, t)] = fq.add(proj_chunk(wqt, qT, t, qb))
                # output rows of q-block qb-1 are complete before island
                # (qb, 0) starts, so these interleave into qb's islands
                for qt in range(4 * (qb - 1), 4 * qb):
                    fq.add(outproj(qt, 0))
                    fq.add(outproj(qt, 1))
            for qb in range(NQB):
                for t in range(4):
                    attn_pair(qb, t, fq, pre[(qb, t)],
                              pv_prereq=pv_pre_00 if (qb, t) == (0, 0)
                              else None)
            fq.drain_all()
            # last q-block's rows only complete after the final island
            for qt in range(12, 16):
                for g in (outproj(qt, 0), outproj(qt, 1)):
                    for _ in g:
                        pass

    nc.compile()
    return nc


def _get_nc():
    if "nc" not in _CACHE:
        _CACHE["nc"] = _build()
    return _CACHE["nc"]


def _run(x, Wq, Wk, Wv, Wo, bo, trace=False, trace_cores=None):
    x = np.asarray(x, dtype=np.float32)
    Wq = np.asarray(Wq, dtype=np.float32)
    Wk = np.asarray(Wk, dtype=np.float32)
    Wv = np.asarray(Wv, dtype=np.float32)
    Wo = np.asarray(Wo, dtype=np.float32)
    bo = np.asarray(bo, dtype=np.float32)

    bf = ml_dtypes.bfloat16
    xTs = [np.ascontiguousarray(x[b].T).astype(bf) for b in range(B)]
    wqs = [np.ascontiguousarray(Wq[:, j * DG:(j + 1) * DG]).astype(bf)
           for j in range(JG)]
    wks = [np.ascontiguousarray(Wk[:, j * DG:(j + 1) * DG]).astype(bf)
           for j in range(JG)]
    wvs = [np.ascontiguousarray(Wv[:, j * DG:(j + 1) * DG]).astype(bf)
           for j in range(JG)]
    wos = [np.ascontiguousarray(Wo[j * DG:(j + 1) * DG, :]).astype(bf)
           for j in range(JG)]

    in_maps = []
    for c in range(8):
        b, j = c // 2, c % 2
        in_maps.append({
            "xT": xTs[b], "wq": wqs[j], "wk": wks[j],
            "wv": wvs[j], "wo": wos[j],
        })

    nc = _get_nc()
    res = run_bass_kernel_spmd(nc, in_maps, list(range(8)), trace=trace,
                               trace_cores=trace_cores)

    full = np.empty((B, S, D), dtype=np.float32)
    for b in range(B):
        full[b] = res.results[2 * b]["out"] + res.results[2 * b + 1]["out"]
    full += bo
    return full, res


def kernel(x, Wq, Wk, Wv, Wo, bo):
    full, _ = _run(x, Wq, Wk, Wv, Wo, bo)
    return full


# revision 32
# speedup vs baseline: 1.0093x; 1.0093x over previous
"""Multi-head causal attention (B=4, S=2048, D=1024, H=16) on 8 Trainium2 cores.

Sharding: core c = (batch b = c//2, head-group j = c%2). Each core computes
one batch and 8 heads (a 512-wide slice of D) entirely on-chip:

  Q^T = Wq_j^T @ x_b^T          [512, 2048]   (bf16, no on-chip transposes)
  K^T = Wk_j^T @ x_b^T          [512, 2048]
  V   = x_b @ Wv_j              [2048, 512]   (k on partitions, + ones column)
  per head h, q-block of 512:
    S^T tile = K_h @ Q_h^T      [128k, 512q]  (contraction over hd=64)
    P^T = exp(S^T / 8)          (no max subtraction needed: scores ~ N(0,1))
    causal: skip/trim tiles above the diagonal, triangle-mask diag tiles
    [ctx^T | rowsum] += [V_h | 1]^T @ P^T     [65, 512] PSUM accumulation
    ctx^T /= rowsum             (reciprocal + partition_broadcast)
  out_partial = ctx^T.T @ Wo_j  [2048, 1024]  (fp32, DMA from PSUM)

Host sums the two head-group partials per batch and adds bo.
"""

import sys

if "/opt/trn_rl_repo" not in sys.path:
    sys.path.insert(0, "/opt/trn_rl_repo")

import numpy as np
import ml_dtypes

import concourse.bacc as bacc
import concourse.tile as tile
from concourse import mybir
from concourse.alu_op_type import AluOpType
from concourse.bass_utils import run_bass_kernel_spmd

B, S, D, H = 4, 2048, 1024, 16
HD = D // H            # 64
JG = 2                 # head-group shards
HPC = H // JG          # 8 heads per core
DG = D // JG           # 512-wide D slice per core
QB = 512               # q-block width
NQB = S // QB          # 4
NKT = S // 128         # 16 k partition-tiles
BF16 = mybir.dt.bfloat16
F32 = mybir.dt.float32
EXP = mybir.ActivationFunctionType.Exp

_CACHE = {}


def _build():
    nc = bacc.Bacc("TRN2", target_bir_lowering=False, debug=False)
    xT = nc.dram_tensor("xT", [D, S], BF16, kind="ExternalInput").ap()
    wq = nc.dram_tensor("wq", [D, DG], BF16, kind="ExternalInput").ap()
    wk = nc.dram_tensor("wk", [D, DG], BF16, kind="ExternalInput").ap()
    wv = nc.dram_tensor("wv", [D, DG], BF16, kind="ExternalInput").ap()
    wo = nc.dram_tensor("wo", [DG, D], BF16, kind="ExternalInput").ap()
    out = nc.dram_tensor("out", [S, D], F32, kind="ExternalOutput").ap()

    xT_r = xT.rearrange("(t p) n -> t p n", p=128)   # [8, 128, 2048]
    wq_r = wq.rearrange("(t p) n -> t p n", p=128)   # [8, 128, 512]
    wk_r = wk.rearrange("(t p) n -> t p n", p=128)
    wv_r = wv.rearrange("(t p) n -> t p n", p=128)
    wo_r = wo.rearrange("(t p) n -> t p n", p=128)   # [4, 128, 1024]

    with tile.TileContext(nc) as tc:
        with (
            tc.tile_pool(name="pers", bufs=1) as pers,
            tc.tile_pool(name="pT", bufs=10) as ppool,
            tc.tile_pool(name="vec", bufs=3) as vpool,
            tc.tile_pool(name="psmm", bufs=2, space="PSUM") as psmm,
            tc.tile_pool(name="pssc", bufs=4, space="PSUM") as pssc,
            tc.tile_pool(name="psctx", bufs=2, space="PSUM") as psctx,
        ):
            xt = [pers.tile([128, S], BF16, tag=f"xt{d}", name=f"xt{d}") for d in range(8)]
            wqt = [pers.tile([128, DG], BF16, tag=f"wq{d}", name=f"wq{d}") for d in range(8)]
            wkt = [pers.tile([128, DG], BF16, tag=f"wk{d}", name=f"wk{d}") for d in range(8)]
            wvt = [pers.tile([128, DG], BF16, tag=f"wv{d}", name=f"wv{d}") for d in range(8)]
            wot = [pers.tile([128, D], BF16, tag=f"wo{d}", name=f"wo{d}") for d in range(4)]
            qT = [pers.tile([128, S], BF16, tag=f"qT{m}", name=f"qT{m}") for m in range(4)]
            kT = [pers.tile([128, S], BF16, tag=f"kT{m}", name=f"kT{m}") for m in range(4)]
            # V with an extra all-ones column per head: [k, 8*(64+1)]
            vt = [pers.tile([128, HPC * (HD + 1)], BF16, tag=f"v{k}", name=f"v{k}")
                  for k in range(NKT)]
            cT = [pers.tile([128, S], BF16, tag=f"cT{m}", name=f"cT{m}") for m in range(4)]
            mask = pers.tile([128, 128], BF16, tag="mask", name="mask")

            # column-chunked x loads, interleaved with the K/Q weights in
            # d-order so the first score-chunk matmul (needs only wkt[0] +
            # xt[0] cols 0:512) starts ~1us in
            for d in range(8):
                nc.sync.dma_start(xt[d][:, 0:512], xT_r[d][:, 0:512])
                nc.sync.dma_start(wkt[d][:], wk_r[d])
                nc.sync.dma_start(wqt[d][:], wq_r[d])
            for d in range(8):
                nc.sync.dma_start(wvt[d][:], wv_r[d])
            for n in range(1, 4):
                for d in range(8):
                    nc.sync.dma_start(xt[d][:, n * 512:(n + 1) * 512],
                                      xT_r[d][:, n * 512:(n + 1) * 512])
            for d in range(4):
                nc.sync.dma_start(wot[d][:], wo_r[d])

            # triangle mask: keep (1.0) where col >= row, else 0
            nc.gpsimd.memset(mask[:], 1.0)
            nc.gpsimd.affine_select(
                out=mask[:], in_=mask[:], compare_op=AluOpType.is_ge,
                fill=0.0, base=0, pattern=[[1, 128]], channel_multiplier=-1,
            )
            for k in range(NKT):
                ones_cols = vt[k].rearrange("p (h c) -> p h c", c=HD + 1)
                nc.gpsimd.memset(ones_cols[:, :, HD:HD + 1], 1.0)

            # ---- emission helpers ----
            # Filler generators emit one PE matmul per yield; the filler
            # queue drip-feeds them into attention islands so projection
            # work rides in the PE slack while ACT (exp) stays saturated.
            def proj_chunk(wt, dst, m, n):
                """One [128,512] tile of Q^T / K^T (m-tile of D-slice rows,
                n-chunk of sequence columns)."""
                ps = psmm.tile([128, 512], F32, tag="mm", name="mm")
                for d in range(8):
                    nc.tensor.matmul(
                        ps[:],
                        wt[d][:, m * 128:(m + 1) * 128],
                        xt[d][:, n * 512:(n + 1) * 512],
                        start=(d == 0), stop=(d == 7),
                    )
                    if d == 7:
                        nc.vector.tensor_copy(
                            dst[m][:, n * 512:(n + 1) * 512], ps[:])
                    yield

            def v_chunk(k):
                """One [128 k-rows, 512] tile of V = x @ Wv (strided into the
                65-column-per-head layout)."""
                ps = psmm.tile([128, 512], F32, tag="mm", name="mm")
                for d in range(8):
                    nc.tensor.matmul(
                        ps[:],
                        xt[d][:, k * 128:(k + 1) * 128],
                        wvt[d][:],
                        start=(d == 0), stop=(d == 7),
                    )
                    if d == 7:
                        nc.vector.tensor_copy(
                            vt[k].rearrange(
                                "p (h c) -> p h c", c=HD + 1)[:, :, 0:HD],
                            ps[:].rearrange("p (h c) -> p h c", c=HD),
                        )
                    yield

            def outproj(qt, n2):
                ps = psmm.tile([128, 512], F32, tag="mm", name="mm")
                for dt in range(4):
                    nc.tensor.matmul(
                        ps[:],
                        cT[dt][:, qt * 128:(qt + 1) * 128],
                        wot[dt][:, n2 * 512:(n2 + 1) * 512],
                        start=(dt == 0), stop=(dt == 3),
                    )
                    if dt == 3:
                        ot = ppool.tile([128, 512], F32, tag="ot", name="ot",
                                        bufs=6)
                        nc.vector.tensor_copy(ot[:], ps[:])
                        nc.sync.dma_start(
                            out[qt * 128:(qt + 1) * 128,
                                n2 * 512:(n2 + 1) * 512],
                            ot[:])
                    yield

            class FillerQueue:
                def __init__(self):
                    self.gens = []
                    self.cur = 0

                def add(self, gen):
                    self.gens.append(gen)
                    return len(self.gens) - 1

                def advance(self, units):
                    while units > 0 and self.cur < len(self.gens):
                        try:
                            next(self.gens[self.cur])
                            units -= 1
                        except StopIteration:
                            self.cur += 1

                def drain_through(self, idx):
                    while self.cur <= idx:
                        try:
                            next(self.gens[self.cur])
                        except StopIteration:
                            self.cur += 1

                def drain_all(self):
                    self.drain_through(len(self.gens) - 1)

            def attn_pair(qb, t, fq, prereq, pv_prereq=None):
                """Attention for head pair (2t, 2t+1) on q-block qb.

                The two heads sit at SBUF partitions 0-63 / 64-127 of tile t,
                so their K=64 score matmuls land on disjoint PE row groups
                and run concurrently. PV matmuls trail by one k-step, and
                ~3 filler (projection) matmuls ride in each step's PE
                slack."""
                fq.drain_through(prereq)
                heads = (2 * t, 2 * t + 1)
                nk = 4 * qb + 4
                ctxs = {h: psctx.tile([65, 512], F32, tag="ctx", name="ctx")
                        for h in heads}

                def score(kt, h):
                    o = max(0, (kt - 4 * qb) * 128)
                    hp = (h % 2) * 64
                    sc = pssc.tile([128, 512], F32, tag="sc", name="sc")
                    nc.tensor.matmul(
                        sc[:, o:],
                        kT[t][hp:hp + 64, kt * 128:(kt + 1) * 128],
                        qT[t][hp:hp + 64, qb * 512 + o:(qb + 1) * 512],
                        start=True, stop=True,
                    )
                    pt = ppool.tile([128, 512], BF16, tag="pT", name="pT")
                    nc.scalar.activation(pt[:, o:], sc[:, o:], EXP,
                                         scale=0.125)
                    if kt >= 4 * qb:
                        nc.vector.tensor_tensor(
                            pt[:, o:o + 128], pt[:, o:o + 128], mask[:],
                            AluOpType.mult)
                    return kt, o, pt

                def pv(h, kt, o, pt):
                    nc.tensor.matmul(
                        ctxs[h][:, o:],
                        vt[kt][:, h * (HD + 1):(h + 1) * (HD + 1)],
                        pt[:, o:],
                        start=(kt == 0), stop=(kt == nk - 1),
                        skip_group_check=True,
                    )

                pend = []
                for kt in range(nk):
                    cur = [(h, *score(kt, h)) for h in heads]
                    pend.append(cur)
                    if len(pend) > 1:
                        if pv_prereq is not None:
                            fq.drain_through(pv_prereq)
                            pv_prereq = None
                        for args in pend.pop(0):
                            pv(*args)
                    fq.advance(1 if qb == 0 else 2)
                for step in pend:
                    for args in step:
                        pv(*args)

                for h in heads:
                    hp = (h % 2) * 64
                    rc = vpool.tile([1, 512], F32, tag="rc", name="rc")
                    nc.vector.reciprocal(rc[:], ctxs[h][64:65, :])
                    bc = vpool.tile([64, 512], F32, tag="bc", name="bc")
                    nc.gpsimd.partition_broadcast(bc[:], rc[:])
                    nc.vector.tensor_tensor(
                        cT[t][hp:hp + 64, qb * 512:(qb + 1) * 512],
                        ctxs[h][0:64, :], bc[:], AluOpType.mult)

            # ---- interleaved emission via the filler queue ----
            fq = FillerQueue()
            pre = {}
            # scores of island (0,0) gate ACT start: K00/Q00 first, V after
            # (the first PV only fires one k-step later)
            fq.add(proj_chunk(wkt, kT, 0, 0))
            pre[(0, 0)] = fq.add(proj_chunk(wqt, qT, 0, 0))
            for k in range(4):
                pv_pre_00 = fq.add(v_chunk(k))
            for t in range(1, 4):
                fq.add(proj_chunk(wkt, kT, t, 0))
                pre[(0, t)] = fq.add(proj_chunk(wqt, qT, t, 0))
            for qb in range(1, NQB):
                fq.add(proj_chunk(wkt, kT, 0, qb))
                q0 = fq.add(proj_chunk(wqt, qT, 0, qb))
                for k in range(4 * qb, 4 * qb + 4):
                    pre[(qb, 0)] = fq.add(v_chunk(k))
                for t in range(1, 4):
                    fq.add(proj_chunk(wkt, kT, t, qb))
                    pre[(qb, t)] = fq.add(proj_chunk(wqt, qT, t, qb))
                # output rows of q-block qb-1 are complete before island
                # (qb, 0) starts, so these interleave into qb's islands
                for qt in range(4 * (qb - 1), 4 * qb):
                    fq.add(outproj(qt, 0))
                    fq.add(outproj(qt, 1))
            for qb in range(NQB):
                for t in range(4):
                    attn_pair(qb, t, fq, pre[(qb, t)],
                              pv_prereq=pv_pre_00 if (qb, t) == (0, 0)
                              else None)
            fq.drain_all()
            # last q-block's rows only complete after the final island
            for qt in range(12, 16):
                for g in (outproj(qt, 0), outproj(qt, 1)):
                    for _ in g:
                        pass

    nc.compile()
    return nc


def _get_nc():
    if "nc" not in _CACHE:
        _CACHE["nc"] = _build()
    return _CACHE["nc"]


def _run(x, Wq, Wk, Wv, Wo, bo, trace=False, trace_cores=None):
    x = np.asarray(x, dtype=np.float32)
    Wq = np.asarray(Wq, dtype=np.float32)
    Wk = np.asarray(Wk, dtype=np.float32)
    Wv = np.asarray(Wv, dtype=np.float32)
    Wo = np.asarray(Wo, dtype=np.float32)
    bo = np.asarray(bo, dtype=np.float32)

    bf = ml_dtypes.bfloat16
    xTs = [np.ascontiguousarray(x[b].T).astype(bf) for b in range(B)]
    wqs = [np.ascontiguousarray(Wq[:, j * DG:(j + 1) * DG]).astype(bf)
           for j in range(JG)]
    wks = [np.ascontiguousarray(Wk[:, j * DG:(j + 1) * DG]).astype(bf)
           for j in range(JG)]
    wvs = [np.ascontiguousarray(Wv[:, j * DG:(j + 1) * DG]).astype(bf)
           for j in range(JG)]
    wos = [np.ascontiguousarray(Wo[j * DG:(j + 1) * DG, :]).astype(bf)
           for j in range(JG)]

    in_maps = []
    for c in range(8):
        b, j = c // 2, c % 2
        in_maps.append({
            "xT": xTs[b], "wq": wqs[j], "wk": wks[j],
            "wv": wvs[j], "wo": wos[j],
        })

    nc = _get_nc()
    res = run_bass_kernel_spmd(nc, in_maps, list(range(8)), trace=trace,
                               trace_cores=trace_cores)

    full = np.empty((B, S, D), dtype=np.float32)
    for b in range(B):
        full[b] = res.results[2 * b]["out"] + res.results[2 * b + 1]["out"]
    full += bo
    return full, res


def kernel(x, Wq, Wk, Wv, Wo, bo):
    full, _ = _run(x, Wq, Wk, Wv, Wo, bo)
    return full
